# revision 37
# baseline (speedup 1.0000x reference)
"""Multi-head attention forward, tensor-parallel over heads across 8 TRN2 cores.

Problem: B=4, S=2048, D=1024, H=16, DK=64.
  qkv = x @ Wqkv.T + bqkv ; per-head scaled-dot-product attention (no mask);
  out = attn_out @ Wout.T + bout

Sharding: 2 heads per core. Work is software-pipelined at q-chunk (512 token)
granularity: iteration i runs scores+exp(i) on PE+ACT while PV(i-1) and
normalize(i-2) interleave into the PE stream as fillers, along with the QKV
projection supertile for the NEXT batch and the output projection for tokens
received from the PREVIOUS batch's AllToAll. ACT (the exp engine) is the
critical resource and is kept saturated; everything else hides behind it.

Key structural points per core:
  - x is pre-transposed / pre-cast to bf16 on the host ([D, T]), so Q^T/K^T
    come out of the QKV matmul feature-major with no PE transposes on x.
  - V' is token-major with a fused ones-column so P@V also yields softmax
    row-sums (PSUM row 64).
  - scores: S^T[tk, tq] = K^T.T @ Q^T, two heads packed as 64-row PE tiles
    (tile_position) writing one 2-bank PSUM slab; one ACT exp op per k-chunk
    covers both heads (scale=1/sqrt(dk)); QC=512 moving dim.
  - normalize: O^T -> PE transpose -> token-major scale by 1/rowsum -> PE
    transpose back to feature-major, so AllToAll receivers can run the out
    projection directly (recv slabs are the matmul lhsT).
  - A tiny warm-up AllToAll issues at kernel start to absorb the one-time
    collective setup / cross-core start skew while early compute runs.
  - Out-proj token ownership: dest core d owns tokens [b*2048 + d*256, +256)
    for every batch b, so each batch forms a complete 8-way AllToAll that
    overlaps the next batch's attention; the host reassembles the result.
"""
import os
import sys

import numpy as np
import ml_dtypes

sys.path.insert(0, "/opt/trn_rl_repo")

import concourse.bass as bass
import concourse.mybir as mybir
import concourse.tile as tile
from concourse import bacc
from concourse.bass_utils import run_bass_kernel_spmd
from concourse.masks import make_identity

F32 = mybir.dt.float32
BF16 = mybir.dt.bfloat16
BF16NP = ml_dtypes.bfloat16

N_CORES = 8
B, S, D, H = 4, 2048, 1024, 16
DK = D // H
T = B * S  # 8192 flattened tokens
HPC = H // N_CORES  # heads per core = 2
FPC = HPC * DK  # features per core = 128
TPC = T // N_CORES  # tokens per core for out-proj = 1024
TPB = TPC // B  # out-proj tokens per core per batch = 256

QC = 512  # q-chunk (moving dim of scores / PV matmuls)
NQC = S // QC  # 4 q-chunks per batch
STT = 512  # QKV token super-tile
TKC = 128  # k-token chunk (partition dim of S^T tiles)
N_TKC = S // TKC  # 16

AluOp = mybir.AluOpType
ActFn = mybir.ActivationFunctionType

_CACHE = {}


def _build():
    nc = bacc.Bacc("TRN2", target_bir_lowering=False, debug=False,
                   num_devices=N_CORES)

    # xtb[st, p, dc*512+t] = x[st*512+t, dc*128+p] (host-tiled x^T, bf16)
    xtb = nc.dram_tensor("xtb", [T // STT, 128, 8 * STT], BF16,
                         kind="ExternalInput")
    # wqkvt[p, dc*384+f] = Wqkv_rows^T[dc*128+p, f]
    wqkvt = nc.dram_tensor("wqkvt", [128, 8 * 3 * FPC], BF16,
                           kind="ExternalInput")
    bqkv3 = nc.dram_tensor("bqkv3", [FPC, 3], F32, kind="ExternalInput")
    # woutt[p, fc*1024+e] = Wout^T[fc*128+p, e]
    woutt = nc.dram_tensor("woutt", [128, 8 * D], BF16, kind="ExternalInput")
    boutr = nc.dram_tensor("boutr", [1, D], F32, kind="ExternalInput")
    y = nc.dram_tensor("y", [TPC, D], F32, kind="ExternalOutput")

    with tile.TileContext(nc) as tc:
        with (
            tc.tile_pool(name="dram", bufs=1, space="DRAM") as dram,
            tc.tile_pool(name="consts", bufs=1) as consts,
            tc.tile_pool(name="qkvt", bufs=2) as qkvt_pool,
            tc.tile_pool(name="vp", bufs=3) as vp_pool,
            tc.tile_pool(name="xt", bufs=2) as xt_pool,
            tc.tile_pool(name="pcomb", bufs=2) as pcomb_pool,
            tc.tile_pool(name="stg", bufs=2) as stg_pool,
            tc.tile_pool(name="sstg", bufs=2) as sstg_pool,
            tc.tile_pool(name="osb", bufs=2) as osb_pool,
            tc.tile_pool(name="yt", bufs=2) as yt_pool,
            tc.tile_pool(name="rcp", bufs=4) as rcp_pool,
            tc.tile_pool(name="s_ps", bufs=2, space="PSUM") as s_ps,
            tc.tile_pool(name="o_ps", bufs=1, space="PSUM") as o_ps,
            tc.tile_pool(name="mm_ps", bufs=1, space="PSUM") as mm_ps,
            tc.tile_pool(name="tr_ps", bufs=1, space="PSUM") as tr_ps,
        ):
            sends = [dram.tile([N_CORES, FPC, TPB], BF16, name=f"send{b}")
                     for b in range(B)]
            recvs = [dram.tile([N_CORES, FPC, TPB], BF16, name=f"recv{b}")
                     for b in range(B)]
            dsend = dram.tile([N_CORES, FPC, TPB], BF16, name="dsend")
            drecv = dram.tile([N_CORES, FPC, TPB], BF16, name="drecv")

            identity = consts.tile([128, 128], BF16)
            make_identity(nc, identity)

            w_sb = consts.tile([128, 8, 3 * FPC], BF16)  # [d_chunk, dc, f]
            nc.scalar.dma_start(out=w_sb, in_=wqkvt[:, :])
            b_sb = consts.tile([FPC, 3], F32)
            nc.scalar.dma_start(out=b_sb, in_=bqkv3[:, :])
            wout_sb = consts.tile([128, 8, D], BF16)  # [f_chunk, fc, e]
            nc.scalar.dma_start(out=wout_sb, in_=woutt[:, :])
            bout_sb = consts.tile([128, D], F32)
            bout_bcast = bass.AP(
                tensor=boutr.ap().tensor,
                offset=boutr.ap().offset,
                ap=[[0, 128], boutr.ap().ap[1]])
            nc.gpsimd.dma_start(out=bout_sb, in_=bout_bcast)

            # warm-up collective: absorbs one-time CC setup + start skew
            nc.gpsimd.collective_compute(
                "AllToAll", AluOp.bypass,
                replica_groups=[list(range(N_CORES))],
                ins=[dsend.opt()], outs=[drecv.opt()])

            qkvts = {}
            vps = {}

            # ---------- filler thunks (PE work interleaved into kc slots) ----
            def qkv_supertile_thunks(b, st):
                # QKV projection for batch b, tokens [st*512, (st+1)*512)
                if b not in qkvts:
                    qkvts[b] = qkvt_pool.tile([128, 3, S], BF16, tag="qkvt",
                                              name=f"qkvt{b}")
                    vps[b] = vp_pool.tile([128, N_TKC, HPC, 66], BF16,
                                          tag="vp", name=f"vp{b}")
                    nc.vector.memset(vps[b][:, :, :, 64:65], 1.0)
                qkvt_b, vp_b = qkvts[b], vps[b]
                sti = b * (S // STT) + st
                xt = xt_pool.tile([128, 8, STT], BF16, tag="xt",
                                  name=f"xt{b}_{st}")

                def load():
                    nc.sync.dma_start(
                        out=xt,
                        in_=xtb[sti].rearrange("p (dc t) -> p dc t", dc=8))

                def proj(fc, t0=0, tn=STT):
                    ps = mm_ps.tile([128, tn], F32, tag="mm",
                                    name=f"qps{b}_{st}_{fc}_{t0}")
                    for dc in range(8):
                        nc.tensor.matmul(
                            ps,
                            w_sb[:, dc, fc * FPC:(fc + 1) * FPC],
                            xt[:, dc, t0:t0 + tn],
                            start=(dc == 0), stop=(dc == 7))
                    nc.vector.tensor_scalar_add(
                        qkvt_b[:, fc, st * STT + t0:st * STT + t0 + tn], ps,
                        b_sb[:, fc:fc + 1])

                def vprep(kc):
                    pst = tr_ps.tile([128, 128], BF16, tag="tr",
                                     name=f"vtr{b}_{kc}")
                    nc.tensor.transpose(
                        pst, qkvt_b[:, 2, kc * TKC:(kc + 1) * TKC], identity)
                    nc.vector.tensor_copy(
                        vp_b[:, kc, :, 0:DK],
                        pst.rearrange("p (h k) -> p h k", h=HPC))

                thunks = [load]
                if b == 0 and st == 0:
                    # pipeline head: K in kc-sized chunks so the first
                    # scores/exp unblock as early as possible
                    thunks += [lambda: proj(0)]
                    thunks += [lambda k=k: proj(1, k * TKC, TKC)
                               for k in range(STT // TKC)]
                    thunks += [lambda: proj(2)]
                else:
                    thunks += [lambda fc=fc: proj(fc) for fc in range(3)]
                thunks += [lambda kc=kc: vprep(kc)
                           for kc in range(st * (STT // TKC),
                                           (st + 1) * (STT // TKC))]
                return thunks

            def pv_tail(b, qc, ops):
                # normalize token-major PV output straight out of PSUM:
                # ops[:, h, tc*65+64] holds the softmax row-sum
                stg = stg_pool.tile([128, QC // 128, HPC, DK], BF16,
                                    tag="stg", name=f"stg{b}_{qc}")
                for h in range(HPC):
                    for tc in range(QC // 128):
                        rcp = rcp_pool.tile([128, 1], F32, tag="rcp",
                                            name=f"rcp{b}_{qc}_{h}_{tc}")
                        nc.vector.reciprocal(
                            rcp, ops[:, h, tc * 65 + DK:tc * 65 + DK + 1])
                        nc.vector.tensor_scalar_mul(
                            stg[:, tc, h, :], ops[:, h, tc * 65:tc * 65 + DK],
                            rcp)
                return stg

            def norm_thunks(b, qc, stg):
                # feature-major transpose-back + send for q-chunk (b, qc)
                sstg = sstg_pool.tile([128, QC], BF16, tag="sstg",
                                      name=f"sstg{b}_{qc}")

                def sendtr(r):
                    pst = tr_ps.tile([128, 128], BF16, tag="tr",
                                     name=f"str{b}_{qc}_{r}")
                    nc.tensor.transpose(pst, stg[:, r, :, :], identity)
                    nc.vector.tensor_copy(sstg[:, r * 128:(r + 1) * 128], pst)

                def ship(j):
                    d = qc * (QC // TPB) + j
                    nc.gpsimd.dma_start(
                        out=sends[b][d],
                        in_=sstg[:, j * TPB:(j + 1) * TPB])

                thunks = [lambda r=r: sendtr(r) for r in range(QC // 128)]
                thunks += [lambda j=j: ship(j) for j in range(QC // TPB)]
                return thunks

            def outproj_thunks(b):
                osb = osb_pool.tile([128, 8, TPB], BF16, tag="osb",
                                    name=f"osb{b}")

                def load():
                    nc.sync.dma_start(
                        out=osb, in_=recvs[b].rearrange("c p t -> p c t"))

                def chunk(tt, ec):
                    yp = mm_ps.tile([128, 512], F32, tag="mm",
                                    name=f"yp{b}_{tt}_{ec}")
                    for fc in range(8):
                        nc.tensor.matmul(
                            yp,
                            osb[:, fc, tt * 128:(tt + 1) * 128],
                            wout_sb[:, fc, ec * 512:(ec + 1) * 512],
                            start=(fc == 0), stop=(fc == 7))
                    yt = yt_pool.tile([128, 512], F32, tag="yt",
                                      name=f"yt{b}_{tt}_{ec}")
                    nc.vector.tensor_add(
                        yt, yp, bout_sb[:, ec * 512:(ec + 1) * 512])
                    nc.sync.dma_start(
                        out=y[b * TPB + tt * 128:b * TPB + (tt + 1) * 128,
                              ec * 512:(ec + 1) * 512],
                        in_=yt)

                return [load] + [lambda tt=tt, ec=ec: chunk(tt, ec)
                                 for tt in range(TPB // 128)
                                 for ec in range(D // 512)]

            # ---------- main per-iteration emission ----------
            def emit_pv_mms(ops, ppcomb, vp_p, kc):
                # PV with pcomb stationary: O[tq, dk+1] token-major, N=65.
                # 8 chains (h x tc) share 2 banks; exactly one start=True per
                # bank marks the whole zero-region pending (HW has_written
                # semantics), every other first-touch overwrites, later MMs
                # accumulate.
                for h in range(HPC):
                    for tc in range(QC // 128):
                        nc.tensor.matmul(
                            ops[:, h, tc * 65:tc * 65 + DK + 1],
                            ppcomb[:, h, kc, tc * 128:(tc + 1) * 128],
                            vp_p[:, kc, h, 0:DK + 1],
                            start=(kc == 0 and tc == 0),
                            stop=(kc == N_TKC - 1),
                            skip_group_check=True)

            def emit_iter(cur, pv_st, fillers):
                """cur=(b,qc) scores+exp; pv_st=(b,qc,pcomb) PV chains
                interleaved per kc; fillers: list of thunks to spread."""
                b, qc = cur
                qkvt_b = qkvts[b]
                q0 = qc * QC
                pcomb = pcomb_pool.tile([128, HPC, N_TKC, QC], BF16,
                                        tag="pc", name=f"pc{b}_{qc}")
                ops = None
                if pv_st is not None:
                    pb, pqc, ppcomb = pv_st
                    vp_p = vps[pb]
                    ops = o_ps.tile([128, HPC, QC], F32, tag="op",
                                    name=f"op{pb}_{pqc}")
                fq = list(fillers)
                fi = 0
                for kc in range(N_TKC):
                    sp = s_ps.tile([128, HPC, QC], F32, tag="sp",
                                   name=f"sp{b}_{qc}_{kc}")
                    for h in range(HPC):
                        kt = qkvt_b[h * DK:(h + 1) * DK, 1,
                                    kc * TKC:(kc + 1) * TKC]
                        qt = qkvt_b[h * DK:(h + 1) * DK, 0, q0:q0 + QC]
                        nc.tensor.matmul(
                            sp[:, h, :], kt, qt,
                            start=True, stop=True,
                            tile_position=(h * DK, 0))
                    nc.scalar.activation(
                        pcomb[:, :, kc, :], sp, ActFn.Exp, scale=1.0 / 8.0)
                    if pv_st is not None:
                        emit_pv_mms(ops, ppcomb, vp_p, kc)
                    # spread filler thunks proportionally across kc slots
                    while fi < len(fq) and fi * N_TKC <= (kc + 1) * len(fq) - N_TKC:
                        fq[fi]()
                        fi += 1
                # normalize PV output, freeing o_ps for the next iteration
                stg = None
                if pv_st is not None:
                    pb, pqc, _ = pv_st
                    stg = pv_tail(pb, pqc, ops)
                # leftover fillers
                while fi < len(fq):
                    fq[fi]()
                    fi += 1
                return pcomb, stg

            # ---------- pipeline ----------
            for st in range(4):
                for t in qkv_supertile_thunks(0, st):
                    t()

            # iteration stream: (b, qc) for all batches
            iters = [(b, qc) for b in range(B) for qc in range(NQC)]
            pv_st = None      # (b, qc, pcomb) awaiting PV
            nm_st = None      # (b, qc, stg) awaiting transpose-back + send
            for idx, (b, qc) in enumerate(iters):
                fillers = []
                if nm_st is not None:
                    nb, nqc, nstg = nm_st
                    fillers += norm_thunks(nb, nqc, nstg)
                    if nqc == 1 and nb >= 1:
                        fillers += outproj_thunks(nb - 1)
                if b + 1 < B:
                    fillers += qkv_supertile_thunks(b + 1, qc)
                pcomb, stg = emit_iter((b, qc), pv_st, fillers)
                if pv_st is not None:
                    pb, pqc, _ = pv_st
                    if pqc == NQC - 1:
                        # batch-last chunk: ship eagerly and trigger the
                        # AllToAll as soon as its data exists
                        for t in norm_thunks(pb, pqc, stg):
                            t()
                        nc.gpsimd.collective_compute(
                            "AllToAll", AluOp.bypass,
                            replica_groups=[list(range(N_CORES))],
                            ins=[sends[pb].opt()], outs=[recvs[pb].opt()])
                        nm_st = None
                    else:
                        nm_st = (pb, pqc, stg)
                pv_st = (b, qc, pcomb)

            # ---------- epilogue ----------
            # PV for the last q-chunk (dense; all exps done)
            b, qc = iters[-1]
            ops = o_ps.tile([128, HPC, QC], F32, tag="op", name="op_last")
            for kc in range(N_TKC):
                emit_pv_mms(ops, pv_st[2], vps[b], kc)
            stg_last = pv_tail(b, qc, ops)
            # ship (b,2) then (b,3), final collective + outproj
            nb, nqc, nstg = nm_st
            for t in norm_thunks(nb, nqc, nstg):
                t()
            for t in norm_thunks(b, qc, stg_last):
                t()
            nc.gpsimd.collective_compute(
                "AllToAll", AluOp.bypass,
                replica_groups=[list(range(N_CORES))],
                ins=[sends[b].opt()], outs=[recvs[b].opt()])
            # keep the PE HAM-warm across the final collective so the last
            # out-projection runs at full clock
            for i in range(70):
                wk = tr_ps.tile([128, 128], BF16, tag="tr", name=f"wk{i}")
                nc.tensor.transpose(wk, identity, identity)
            for t in outproj_thunks(B - 1):
                t()

    nc.compile()
    return nc


def _get_nc():
    if "nc" not in _CACHE:
        _CACHE["nc"] = _build()
    return _CACHE["nc"]


def kernel(x, Wqkv, bqkv, Wout, bout):
    x = np.asarray(x, dtype=np.float32)
    Wqkv = np.asarray(Wqkv, dtype=np.float32)
    bqkv = np.asarray(bqkv, dtype=np.float32)
    Wout = np.asarray(Wout, dtype=np.float32)
    bout = np.asarray(bout, dtype=np.float32)

    # tiled x^T: xtb[st, p, dc, t] = x[st*512+t, dc*128+p], 8KB DMA lines
    xtb = np.ascontiguousarray(
        x.reshape(T // STT, STT, 8, 128).transpose(0, 3, 2, 1)
        .astype(BF16NP).reshape(T // STT, 128, 8 * STT))
    # tiled Wout^T: woutt[p, fc, e] = Wout.T[fc*128+p, e]
    woutt = np.ascontiguousarray(
        Wout.T.reshape(8, 128, D).transpose(1, 0, 2)
        .astype(BF16NP).reshape(128, 8 * D))
    boutr = bout.reshape(1, D)

    in_maps = []
    for c in range(N_CORES):
        f0 = c * FPC  # first feature row of this core's heads
        rows = np.concatenate([
            Wqkv[f0:f0 + FPC],                  # q rows
            Wqkv[D + f0:D + f0 + FPC],          # k rows
            Wqkv[2 * D + f0:2 * D + f0 + FPC],  # v rows
        ])  # [384, 1024]
        # tiled: wqkvt[p, dc, f] = rows.T[dc*128+p, f]
        wqkvt = np.ascontiguousarray(
            rows.T.reshape(8, 128, 3 * FPC).transpose(1, 0, 2)
            .astype(BF16NP).reshape(128, 8 * 3 * FPC))
        bq = np.concatenate([
            bqkv[f0:f0 + FPC],
            bqkv[D + f0:D + f0 + FPC],
            bqkv[2 * D + f0:2 * D + f0 + FPC],
        ])  # [384]
        bqkv3 = np.ascontiguousarray(bq.reshape(3, FPC).T)  # [128, 3]
        in_maps.append({
            "xtb": xtb,
            "wqkvt": wqkvt,
            "bqkv3": bqkv3,
            "woutt": woutt,
            "boutr": boutr,
        })

    nc = _get_nc()
    trace = os.environ.get("MHA_TRACE") == "1"
    res = run_bass_kernel_spmd(
        nc, in_maps, core_ids=list(range(N_CORES)), trace=trace)
    if trace:
        _CACHE["last_result"] = res

    # y_c[b*256 + r] holds global token b*2048 + c*256 + r
    ys = np.stack([res.results[c]["y"].reshape(B, TPB, D)
                   for c in range(N_CORES)], axis=1)  # [B, core, TPB, D]
    return np.ascontiguousarray(ys.reshape(B, S, D))
